# revision 52
# baseline (speedup 1.0000x reference)
"""Causal self-attention (B=4, T=2048, C=1024, H=16, D=64) on 8 Trainium2 cores.

Sharding: core c = (b, hg) with b = c // 2 (batch), hg = c % 2 (head-group of
8 heads = 512 of 1024 qkv columns). Each core computes q/k/v projections for
its (b, hg), causal attention for its 8 heads, and a partial output
projection y_hg @ Wp[hg]. Host sums the two head-group partials per batch and
adds the projection bias.

Per-core kernel (scores/AV matmuls bf16, q/k/v projections in fp8e4m3
split-precision DoubleRow, PSUM accumulation + softmax in fp32):

  - q/k/v projections use the PE's fp8 DoubleRow mode (2 contraction k-tiles
    per instruction at 0.5 cycles/output-column = 4x the bf16 MAC rate).
    Accuracy is preserved with a 3-term split: host supplies xh = fp8(4*x),
    xl = fp8(4*x - xh) and Wh = fp8(S*W), Wl = fp8(S*W - Wh); the kernel
    accumulates xh@Wh + xh@Wl + xl@Wh in one PSUM group (the dropped xl@Wl
    term is ~0.05% of the result, below bf16 rounding). The descale constant
    1/(4*S) folds into the existing PSUM->SBUF bias-add (dual-scalar
    tensor_scalar / scalar_tensor_tensor), so DVE work is unchanged. This is
    MORE accurate than the bf16 baseline (measured) and 25% cheaper on PE.
  - qT/kT [col, t] via lhsT = weight chunk, rhs = xT chunk; v natural [t, col]
    via lhsT = xT chunk, rhs = Wv; v is stored in 65-wide groups per head with
    a ones-column.
  - scoresT chunks [s=128, t<=512] on PE, block-diagonal chunks packed
    compactly so one ScalarE exp instruction covers each chunk pair (no
    max-subtraction: logits are ~N(0,1); fp32 exp cannot overflow). Causal
    zeroing of diagonal chunks multiplies by a precomputed 0/1 mask (DVE).
  - AV runs TRANSPOSED for full PE-array utilization: lhsT = att chunk
    [s=128, t=128] (stationary), rhs = [v | ones] [s=128, 65] (moving),
    accumulating yz[t=128, 65] per (head, t-chunk) in PSUM; column 64 is the
    softmax denominator Z. One accumulation group per head (PSUM allows one
    open group per 2KB bank; first write per address overwrites). Rows are
    scaled by 1/Z into y natural [t, col] (bf16), then PE-transposed back to
    [col, t] for the output projection.

Scheduling: the attention stream is software-pipelined -- scores+exp of
chunk-pair i+1 are emitted BEFORE mask+AV of pair i, so ScalarE's exp always
overlaps PE work (the PE wait queue releases in order, so anything emitted
behind a parked AV cannot run early). Projection matmuls ride in two filler
queues consumed between pipeline stages: `gated` (q/k/v projections, whose
execution also gates attention heads via per-pair markers) and `reserve`
(output projections, saved for the exp-heaviest last quarter; Q1/Q2 quotas
deliberately under-draw so leftovers spill into Q3's exp shadow). Quarter 0
defers ALL its mask/AV work past the whole startup stream (q/k operands
first, v operands behind) so the exp train starts as soon as x+q/k weights
land (~12us); the deferred AVs flush under quarter 1's exp shadow. The final
projection opens all 8 PSUM output groups at once (two share each 2-bank
score buf -- accumulation groups are per-bank) so only the last pair's 8
closing matmuls wait on the final transpose, and the drain's psum->bf16
copies and output DMAs fan out over DVE/ACT and the SP/ACT/Pool DMA queues.
"""

import sys

if "/opt/trn_rl_repo" not in sys.path:
    sys.path.insert(0, "/opt/trn_rl_repo")

from contextlib import ExitStack

import numpy as np

import concourse.mybir as mybir
import concourse.tile as tile
from concourse import bacc, masks
from concourse.bass_utils import run_bass_kernel_spmd

F32 = mybir.dt.float32
BF16 = mybir.dt.bfloat16
FP8 = mybir.dt.float8e4
DR = mybir.MatmulPerfMode.DoubleRow
AF = mybir.ActivationFunctionType
MULT = mybir.AluOpType.mult
ADD = mybir.AluOpType.add

# split-precision scales: x staged as fp8(4x) + residual; weights staged as
# fp8(S*W) + residual with S chosen so values sit ~N(0,1) in e4m3 range.
# PSUM comes out 4*S too large; descale folds into the bias-add.
XS = 4.0
QS = 256.0   # wq carries the 1/sqrt(D) attention scale: sigma(256*Wq/8) = 1
KS = 32.0
VS = 32.0
QDESC = 1.0 / (XS * QS)
KDESC = 1.0 / (XS * KS)
VDESC = 1.0 / (XS * VS)

C = 1024      # embed dim
T = 2048      # sequence length
B = 4         # batch
NCOL = 512    # qkv columns per core (8 heads x 64)
TB = 512      # t-block / quarter size
SC = 128      # s-chunk size
D = 64        # head dim

N_WARM = 12  # PE clock-ramp dummy matmuls during startup DMA

LAST_RESULTS = None  # BassKernelResults of the most recent run (for test.py)
TRACE = False


def _build():
    N_PAIRS = NCOL // 128          # head-pairs per core (4)
    CC = C // 128                  # contraction chunks (8)
    N_TB = T // TB                 # t-blocks / quarters (4)
    SPB = TB // SC                 # s-chunks per t-block (4)
    N_TT = TB // SC                # t-subchunks per t-block (4)
    VGRP = 2 * N_PAIRS             # head groups in v_buf (8)
    VROW = VGRP * 65               # 520

    nc = bacc.Bacc("TRN2", target_bir_lowering=False, debug=False)

    xTh = nc.dram_tensor("xTh", (C, T), FP8, kind="ExternalInput")
    xTl = nc.dram_tensor("xTl", (C, T), FP8, kind="ExternalInput")
    wqh = nc.dram_tensor("wqh", (C, NCOL), FP8, kind="ExternalInput")
    wql = nc.dram_tensor("wql", (C, NCOL), FP8, kind="ExternalInput")
    wkh = nc.dram_tensor("wkh", (C, NCOL), FP8, kind="ExternalInput")
    wkl = nc.dram_tensor("wkl", (C, NCOL), FP8, kind="ExternalInput")
    wvh = nc.dram_tensor("wvh", (C, NCOL), FP8, kind="ExternalInput")
    wvl = nc.dram_tensor("wvl", (C, NCOL), FP8, kind="ExternalInput")
    wp = nc.dram_tensor("wp", (NCOL, C), BF16, kind="ExternalInput")
    bqk = nc.dram_tensor("bqk", (NCOL, 2), F32, kind="ExternalInput")
    bv = nc.dram_tensor("bv", (1, NCOL), BF16, kind="ExternalInput")
    out = nc.dram_tensor("out", (T, C), BF16, kind="ExternalOutput")

    with tile.TileContext(nc) as tc, ExitStack() as ctx:
        const = ctx.enter_context(tc.tile_pool(name="const", bufs=1))
        xq_pool = ctx.enter_context(tc.tile_pool(name="xq", bufs=4))
        w_pool = ctx.enter_context(tc.tile_pool(name="wqkv", bufs=1))
        qt_pool = ctx.enter_context(tc.tile_pool(name="qt", bufs=2))
        att_pool = ctx.enter_context(tc.tile_pool(name="att", bufs=17))
        ynat_pool = ctx.enter_context(tc.tile_pool(name="ynat", bufs=2))
        yt_pool = ctx.enter_context(tc.tile_pool(name="yt", bufs=4))
        small = ctx.enter_context(tc.tile_pool(name="small", bufs=2))
        ostage = ctx.enter_context(tc.tile_pool(name="ostage", bufs=6))
        ps_acc = ctx.enter_context(tc.tile_pool(name="ps_acc", bufs=2, space="PSUM"))
        ps1 = ps_acc
        ps_po = ps_acc
        ps_sc = ctx.enter_context(tc.tile_pool(name="ps_sc", bufs=2, space="PSUM"))
        ps_yz = ctx.enter_context(tc.tile_pool(name="ps_yz", bufs=2, space="PSUM"))

        kT = const.tile([128, N_PAIRS * T], BF16, tag="kT")   # [col_in_pair, p*T + s]
        v_buf = const.tile([128, (T // SC) * VROW], BF16, tag="vbuf")
        wp_sb = const.tile([128, N_PAIRS * C], BF16, tag="wp")
        bqk_sb = const.tile([128, 2 * N_PAIRS], F32, tag="bqk")
        bv_sb = const.tile([1, NCOL], BF16, tag="bv")
        wq_h = w_pool.tile([128, CC * NCOL], FP8, tag="wqh")
        wq_l = w_pool.tile([128, CC * NCOL], FP8, tag="wql")
        wk_h = w_pool.tile([128, CC * NCOL], FP8, tag="wkh")
        wk_l = w_pool.tile([128, CC * NCOL], FP8, tag="wkl")
        wv_h = w_pool.tile([128, CC * NCOL], FP8, tag="wvh")
        wv_l = w_pool.tile([128, CC * NCOL], FP8, tag="wvl")

        # Startup DMAs, all on ONE queue (the DMA device is serial and its
        # cross-queue arbitration is request-order FIFO, so a single queue is
        # the only way to get a deterministic priority order). The exp train
        # is the startup critical path, so q/k operands stream FIRST (x0h,
        # then hi-weight column halves for pairs 0/1, full lo weights), then
        # the v operands (consumed as deferred filler), then quarter 1's x
        # prefetch, then wp (first needed ~150us in).
        xh_tiles = {}
        x0h = xq_pool.tile([128, CC * TB], FP8, tag="xh", name="xh0")
        x0l = xq_pool.tile([128, CC * TB], FP8, tag="xl", name="xl0")
        xh_tiles[0] = (x0h, x0l)
        x1h = xq_pool.tile([128, CC * TB], FP8, tag="xh", name="xh1")
        x1l = xq_pool.tile([128, CC * TB], FP8, tag="xl", name="xl1")
        xh_tiles[1] = (x1h, x1l)
        xsrc_h = xTh.ap()[:, 0:TB].rearrange("(cc a) t -> a cc t", a=128)
        xsrc_l = xTl.ap()[:, 0:TB].rearrange("(cc a) t -> a cc t", a=128)
        wvh_src = wvh.ap().rearrange("(cc a) n -> a cc n", a=128)
        wvl_src = wvl.ap().rearrange("(cc a) n -> a cc n", a=128)
        nc.sync.dma_start(
            x0h[:].rearrange("a (cc t) -> a cc t", cc=CC), xsrc_h[:]
        )
        # hi q/k weights in column halves (fp8 256B rows pay the 2x DMA
        # latency penalty but pairs 0/1 start ~4us earlier); lo weights as
        # single full-width transfers (512B rows, full rate)
        HC = NCOL // 2
        def wdma(wsb, wsrc, half):
            nc.sync.dma_start(
                wsb[:].rearrange("a (cc n) -> a cc n", cc=CC)[
                    :, :, half * HC : (half + 1) * HC
                ],
                wsrc.ap().rearrange("(cc a) n -> a cc n", a=128)[
                    :, :, half * HC : (half + 1) * HC
                ],
            )
        wdma(wq_h, wqh, 0)
        nc.sync.dma_start(
            wq_l[:].rearrange("a (cc n) -> a cc n", cc=CC),
            wql.ap().rearrange("(cc a) n -> a cc n", a=128),
        )
        # x0l immediately after the q weights: the q/k units' third split
        # term (xl@Wh) gates the first scores
        nc.sync.dma_start(
            x0l[:].rearrange("a (cc t) -> a cc t", cc=CC), xsrc_l[:]
        )
        # merged q/k biases in ONE transfer (was two)
        nc.sync.dma_start(
            bqk_sb[:].rearrange("a (p o) -> a p o", o=2),
            bqk.ap().rearrange("(p a) o -> a p o", a=128),
        )
        wdma(wk_h, wkh, 0)
        nc.sync.dma_start(
            wk_l[:].rearrange("a (cc n) -> a cc n", cc=CC),
            wkl.ap().rearrange("(cc a) n -> a cc n", a=128),
        )
        wdma(wq_h, wqh, 1)
        wdma(wk_h, wkh, 1)
        nc.sync.dma_start(
            wv_h[:].rearrange("a (cc n) -> a cc n", cc=CC), wvh_src[:]
        )
        nc.sync.dma_start(bv_sb[:], bv.ap())
        nc.sync.dma_start(
            wv_l[:].rearrange("a (cc n) -> a cc n", cc=CC), wvl_src[:]
        )
        nc.sync.dma_start(
            x1h[:].rearrange("a (cc t) -> a cc t", cc=CC),
            xTh.ap()[:, TB : 2 * TB].rearrange("(cc a) t -> a cc t", a=128),
        )
        nc.sync.dma_start(
            x1l[:].rearrange("a (cc t) -> a cc t", cc=CC),
            xTl.ap()[:, TB : 2 * TB].rearrange("(cc a) t -> a cc t", a=128),
        )
        # wp is not needed until the first output projection (~60us in)
        nc.sync.dma_start(
            wp_sb[:].rearrange("a (p n) -> a p n", p=N_PAIRS),
            wp.ap().rearrange("(p a) n -> a p n", a=128),
        )
        # 0/1 causal triangle mask: msk[s, f] = (f >= s); block-diagonal
        # offset r uses the width-(TB - r*SC) prefix of the same tile.
        # memset to 1.0 first (cheap, DMA-independent) so the PE warm-up can
        # start immediately; the triangle select rewrites it in place after.
        msk = const.tile([128, TB], BF16, tag="msk")
        nc.vector.memset(msk[:, 0:256], 1.0)
        # PE warm-up: dummy matmuls on the DMA-independent mask tile keep the
        # PE clock ramped while the input DMAs stream; a guard read into an
        # unused cell keeps them alive through DCE
        warm_ps = ps_sc.tile([128, 2 * TB], F32, tag="st", name="warm_ps")
        for _ in range(N_WARM):
            nc.tensor.matmul(
                warm_ps[:, 0:256], msk[:, 0:128], msk[:, 0:256], start=True, stop=True
            )
        nc.vector.memset(msk[:, 256:TB], 1.0)
        nc.gpsimd.affine_select(
            out=msk[:],
            in_=msk[:],
            compare_op=mybir.AluOpType.is_ge,
            fill=0.0,
            base=0,
            channel_multiplier=-1,
            pattern=[[1, TB]],
        )
        guard = const.tile([1, 1], BF16, tag="guard")
        nc.vector.tensor_copy(guard[:], warm_ps[0:1, 0:1])
        nc.sync.dma_start(out.ap()[0:1, 0:1], guard[:])
        ones_f32 = const.tile([128, max(128, (T // SC) * VGRP)], F32, tag="ones_f32")
        nc.vector.memset(ones_f32[:], 1.0)
        # bv broadcast across partitions once: the v-units then fold the bias
        # into their psum->SBUF copy instead of spending a PE matmul each
        bv_bc = const.tile([128, NCOL], BF16, tag="bv_bc")
        nc.gpsimd.partition_broadcast(bv_bc[:], bv_sb[:])
        # ones columns of v_buf (col 64 of each 65-group)
        nc.vector.tensor_copy(
            v_buf[:].rearrange("a (t g o) -> a t g o", g=VGRP, o=65)[:, :, :, 64:65],
            ones_f32[:, : (T // SC) * VGRP].rearrange("a (t g) -> a t g", g=VGRP)[
                :, :, :, None
            ],
        )

        # per-pair filler draw: sized to the exp-vs-PE deficit of each quarter
        # so earlier quarters don't starve the ACT-bound last quarter
        # counts are in thunks; DR projection thunks are half the duration of
        # the baseline's bf16 thunks. Q0-Q2 pace the (cheap, 107ns) DR qkv
        # thunks; Q3's draws are deficit-sized for the (213ns) reserve proj
        # thunks so the reserve lasts through the exp-heavy last quarter.
        FILL_QUOTA = {
            0: [12, 12],
            1: [5, 5, 5, 5],
            2: [4, 4, 3, 3, 2, 2],
            3: [2, 0, 0, 2, 2, 2, 2, 2],
        }

        def head_units(tb, p, h):
            """Chunk-pair units of one attention head, for the global
            score/exp -> mask/AV software pipeline."""
            n_chunk = SPB * tb + SPB
            # diagonal chunks first: their exp->mask chain then overlaps with
            # the plain chunks' matmuls instead of stalling AV
            if tb > 0:
                j_order = (
                    [0, 1]
                    + list(range(SPB * tb, n_chunk))
                    + list(range(2, SPB * tb))
                )
            else:
                j_order = list(range(n_chunk))
            # per t-subchunk: positions in j_order of the first/last
            # contributing s-chunk (j contributes to tt iff j - SPB*tb <= tt)
            first_idx = {}
            last_idx = {}
            for idx, j in enumerate(j_order):
                r = j - SPB * tb
                for tt in range(max(r, 0), N_TT):
                    if tt not in first_idx:
                        first_idx[tt] = idx
                    last_idx[tt] = idx
            return [
                dict(
                    tb=tb, p=p, h=h, jj=jj, j_order=j_order, n_chunk=n_chunk,
                    first=first_idx, last=last_idx,
                )
                for jj in range(0, n_chunk, 2)
            ]

        def av_start_stop(u, idx, r, tt):
            """PSUM allows only ONE open accumulation group per 2KB bank
            (start marks the whole zero-region pending): the whole head's AV
            accumulation is a single group; the first write to each address
            overwrites, later writes accumulate."""
            r0 = u["j_order"][0] - SPB * u["tb"]
            start = idx == 0 and tt == max(r0, 0)
            stop = idx == u["n_chunk"] - 1 and tt == N_TT - 1
            return start, stop

        def score_exp(u):
            """Stage 1: score matmuls + exp for one chunk pair."""
            tb, p, h, jj = u["tb"], u["p"], u["h"], u["jj"]
            hrow = h * 64
            qT = qt_tiles[tb]
            st = ps_sc.tile([128, 2 * TB], F32, tag="st")
            at = att_pool.tile([128, 2 * TB], BF16, tag="at")
            # chunks are packed compactly (chunk k at offset o_k, width
            # TB - c0_k) so one exp instruction covers the whole pair
            cols = []
            o = 0
            for k in range(2):
                j = u["j_order"][jj + k]
                r = j - SPB * tb  # >=0 only for block-diag chunks
                c0 = max(0, r * SC)  # first valid t-col
                cols.append((jj + k, j, r, c0, o))
                nc.tensor.matmul(
                    st[:, o : o + TB - c0],
                    kT[hrow : hrow + 64, p * T + j * SC : p * T + j * SC + SC],
                    qT[hrow : hrow + 64, p * TB + c0 : (p + 1) * TB],
                    start=True,
                    stop=True,
                )
                o += TB - c0
            nc.scalar.activation(at[:, 0:o], st[:, 0:o], AF.Exp)
            u["at"] = at
            u["cols"] = cols

        head_yz = {}

        def mask_av(u):
            """Stage 2: causal mask + transposed-AV accumulation; emits the
            head's normalize after its last pair."""
            tb, p, h, jj = u["tb"], u["p"], u["h"], u["jj"]
            g = 2 * p + h
            at = u["at"]
            if jj == 0:
                head_yz[(tb, g)] = ps_yz.tile(
                    [128, N_TT * 65], F32, tag="yz", name=f"yz_{tb}_{g}"
                )
            yz = head_yz[(tb, g)]
            for idx, j, r, c0, o in u["cols"]:
                if r >= 0:
                    # zero att where t_loc < r*SC + s_loc
                    nc.vector.tensor_mul(
                        at[:, o : o + TB - c0],
                        at[:, o : o + TB - c0],
                        msk[:, 0 : TB - c0],
                    )
                vj = v_buf[:, j * VROW + g * 65 : j * VROW + g * 65 + 65]
                # transposed AV: att chunk stationary, [v | ones] moving;
                # accumulates yz[t, 0:64] = y and yz[t, 64] = Z
                for tt in range(max(r, 0), N_TT):
                    start, stop = av_start_stop(u, idx, r, tt)
                    nc.tensor.matmul(
                        yz[:, tt * 65 : tt * 65 + 65],
                        at[:, o + tt * SC - c0 : o + tt * SC - c0 + SC],
                        vj,
                        start=start,
                        stop=stop,
                    )
            if jj == u["n_chunk"] - 2:
                # normalize: y[t, d] / Z[t] for all 4 t-subchunks at once
                yzv = yz[:].rearrange("a (tt o) -> a tt o", o=65)
                rz = small.tile([128, N_TT], F32, tag="rz")
                nc.vector.reciprocal(rz[:][:, :, None], yzv[:, :, 64:65])
                ynat = ynat_tiles[tb]
                # pair-major ynat layout [t, (p, tt, 128)]: each pair's block
                # is contiguous, so ONE xbar DMA transposes it later
                nc.vector.tensor_mul(
                    ynat[:].rearrange(
                        "a (p tt c) -> a p tt c", p=N_PAIRS, tt=N_TT
                    )[:, p, :, h * 64 : h * 64 + 64],
                    yzv[:, :, 0:64],
                    rz[:][:, :, None].broadcast_to((128, N_TT, 64)),
                )
                head_yz.pop((tb, g))

        def emit_transpose(tb, p):
            """Transpose ynat[t, cols of pair p] -> yt[col, t] on the DMA
            xbar (out[c, tt, t] = in[t, tt*128+c]): one call per pair, 14ns
            per 16x128 tile on the ~17%-busy DMA device -- no PE matmuls, no
            DVE copies, no PSUM. Emitted a half-head after the normalize so
            the queue never parks on the input semaphore."""
            ynat = ynat_tiles[tb]
            yt = yt_tiles[tb]
            nc.sync.dma_start_transpose(
                yt[:, p * TB : (p + 1) * TB].rearrange(
                    "a (tt t) -> a tt t", tt=N_TT
                ),
                ynat[:, p * (N_TT * 128) : (p + 1) * (N_TT * 128)],
            )

        qt_tiles = {}
        ynat_tiles = {}
        yt_tiles = {}

        NJ = CC // 2  # DR k-tile pairs per contraction (4)

        def qkv_thunks(tb):
            """Per-matmul thunks for quarter tb's projections, to be spliced
            one-at-a-time into the attention stream of quarter tb-1.
            Ordered v-first, then (q, k) per pair, so att(tb) head pairs can
            start as soon as their own pair's projections are consumed.

            Each unit is 12 fp8 DoubleRow matmuls: 3 split terms (xh@Wh,
            xh@Wl, xl@Wh) x 4 k-tile pairs, one PSUM accumulation group."""
            thunks = []
            t0 = tb * TB
            xh8, xl8 = xh_tiles[tb]
            # (x tile, use-lo-weight): term order; start on the first, stop
            # on the last emitted matmul of the group
            TERMS = ((xh8, False), (xh8, True), (xl8, False))

            def wap(wsb, j, c0, c1):
                return wsb[:].rearrange("a (cc n) -> a cc n", cc=CC)[
                    :, 2 * j : 2 * j + 2, c0:c1
                ]

            def xap(xt, j, c0, c1):
                return xt[:].rearrange("a (cc t) -> a cc t", cc=CC)[
                    :, 2 * j : 2 * j + 2, c0:c1
                ]

            v_units = []
            for tth in range(TB // 128):
                tt = (t0 // 128) + tth
                pt_box = [None]
                # quarter 0 runs its v-units chunk-major behind the startup
                # DMA stream; units 2/3 borrow idle score-pool banks so all
                # four accumulation groups can be open at once
                vpool, vtag = (
                    (ps_yz, "yz") if tb == 0 and tth >= 2 else (ps1, "acc")
                )
                unit = []
                for ti, (xt, lo) in enumerate(TERMS):
                    wt = wv_l if lo else wv_h
                    def mkv(ti=ti, j=None, xt=xt, wt=wt, tth=tth, tt=tt,
                            pt_box=pt_box, vpool=vpool, vtag=vtag):
                        def go():
                            if ti == 0 and j == 0:
                                pt_box[0] = vpool.tile(
                                    [128, NCOL], F32, tag=vtag, name=f"psv_{tb}_{tth}"
                                )
                            pt = pt_box[0]
                            nc.tensor.matmul(
                                pt[:],
                                xap(xt, j, tth * 128, tth * 128 + 128),
                                wap(wt, j, 0, NCOL),
                                start=(ti == 0 and j == 0),
                                stop=(ti == 2 and j == NJ - 1),
                                perf_mode=DR,
                            )
                            if ti == 2 and j == NJ - 1:
                                nc.vector.scalar_tensor_tensor(
                                    v_buf[:, tt * VROW : (tt + 1) * VROW].rearrange(
                                        "a (g o) -> a g o", g=VGRP
                                    )[:, :, 0:64],
                                    pt[:].rearrange("a (g o) -> a g o", g=VGRP),
                                    VDESC,
                                    bv_bc[:].rearrange("a (g o) -> a g o", g=VGRP),
                                    MULT,
                                    ADD,
                                )
                        return go
                    unit.extend(mkv(j=j) for j in range(NJ))
                v_units.append(unit)
            v_thunks = []
            if tb == 0:
                # chunk-major: thunk i across units so each landed DMA chunk
                # group unblocks all four units' next matmul
                for i in range(3 * NJ):
                    for un in v_units:
                        v_thunks.append(un[i])
            else:
                for un in v_units:
                    v_thunks.extend(un)
            if tb > 0:
                # steady state: v first, then (q, k) per pair
                thunks.extend(v_thunks)
            for u in range(2 * N_PAIRS):
                p, which = u // 2, u % 2
                wh_sb, wl_sb, desc = (
                    (wq_h, wq_l, QDESC),
                    (wk_h, wk_l, KDESC),
                )[which]
                dst = (
                    qt_tiles[tb][:, p * TB : (p + 1) * TB]
                    if which == 0
                    else kT[:, p * T + t0 : p * T + t0 + TB]
                )
                pt_box = [None]
                for ti, (xt, lo) in enumerate(TERMS):
                    wt = wl_sb if lo else wh_sb
                    def mk(ti=ti, j=None, xt=xt, wt=wt, u=u, p=p, which=which,
                           desc=desc, dst=dst, pt_box=pt_box):
                        def go():
                            if ti == 0 and j == 0:
                                pt_box[0] = ps1.tile(
                                    [128, TB], F32, tag="acc", name=f"ps_{tb}_{u}"
                                )
                            pt = pt_box[0]
                            nc.tensor.matmul(
                                pt[:],
                                wap(wt, j, p * 128, p * 128 + 128),
                                xap(xt, j, 0, TB),
                                start=(ti == 0 and j == 0),
                                stop=(ti == 2 and j == NJ - 1),
                                perf_mode=DR,
                            )
                            if ti == 2 and j == NJ - 1:
                                nc.vector.tensor_scalar(
                                    dst, pt[:], desc,
                                    bqk_sb[:, 2 * p + which : 2 * p + which + 1],
                                    MULT, ADD,
                                )
                        return go
                    thunks.extend(mk(j=j) for j in range(NJ))
            if tb == 0:
                # quarter 0: q/k pairs FIRST so the exp train starts as soon
                # as the q/k weights land; the v block (startup-DMA paced)
                # rides behind as filler, with all tb0 AVs deferred past it
                thunks.extend(v_thunks)
            return thunks

        def proj_thunks(tb, borrow=False):
            """Per-matmul thunks for t-block tb's output projection. With
            borrow=True (the final projection, after all attention), half the
            accumulator groups use the idle score pool's slots so four
            groups pipeline instead of two."""
            t0 = tb * TB
            yt = yt_tiles[tb]
            groups = []
            sc_box = {}
            for tt in range(TB // 128):
                for nh in range(C // 512):
                    gi = tt * (C // 512) + nh
                    po_box = [None]
                    def mk(p, tt=tt, nh=nh, gi=gi, po_box=po_box):
                        def go():
                            if p == 0:
                                if borrow:
                                    # 8 concurrently-open groups: acc 2 +
                                    # yz 2 + two 2-bank sc bufs holding TWO
                                    # 512-col groups each (PSUM accumulation
                                    # groups are per-bank, so the two halves
                                    # are independent groups)
                                    kind = (0, 1, 1, 2, 0, 1, 1, 0)[gi]
                                    if kind == 1:
                                        if gi in (1, 5):
                                            sc_box[gi] = ps_sc.tile(
                                                [128, 1024], F32, tag="st",
                                                name=f"po_{tb}_{gi}",
                                            )
                                            po_box[0] = sc_box[gi][:, 0:512]
                                        else:
                                            po_box[0] = sc_box[gi - 1][:, 512:1024]
                                    elif kind == 2:
                                        po_box[0] = ps_yz.tile(
                                            [128, 512], F32, tag="yz",
                                            name=f"po_{tb}_{tt}_{nh}",
                                        )[:]
                                    else:
                                        po_box[0] = ps_po.tile(
                                            [128, 512], F32, tag="acc",
                                            name=f"po_{tb}_{tt}_{nh}",
                                        )[:]
                                else:
                                    po_box[0] = ps_po.tile(
                                        [128, 512], F32, tag="acc",
                                        name=f"po_{tb}_{tt}_{nh}",
                                    )[:]
                            po = po_box[0]
                            nc.tensor.matmul(
                                po,
                                yt[:, p * TB + tt * 128 : p * TB + tt * 128 + 128],
                                wp_sb[:, p * C + nh * 512 : p * C + nh * 512 + 512],
                                start=(p == 0),
                                stop=(p == N_PAIRS - 1),
                            )
                            if p == N_PAIRS - 1:
                                ob = ostage.tile([128, 512], BF16, tag="ob")
                                # psum->bf16 staging: copies split DVE/ACT
                                # (GPSIMD cannot read PSUM); each group's DMA
                                # issues from the queue OPPOSITE its copy
                                # engine so the four queues pipeline the tail
                                if borrow and gi % 2:
                                    nc.scalar.copy(ob[:], po)
                                else:
                                    nc.vector.tensor_copy(ob[:], po)
                                dq = (
                                    (nc.scalar, nc.sync, nc.gpsimd)[gi % 3]
                                    if borrow
                                    else nc.sync
                                )
                                dq.dma_start(
                                    out.ap()[
                                        t0 + tt * 128 : t0 + tt * 128 + 128,
                                        nh * 512 : (nh + 1) * 512,
                                    ],
                                    ob[:],
                                )
                        return go
                    groups.append([mk(p) for p in range(N_PAIRS)])
            if not borrow:
                return [th for g in groups for th in g]
            # Drain order: the last pair's yt block (p=3) depends on the
            # final transpose of the attention stream, so emit p0..p2 of all
            # eight concurrently-open PSUM groups first -- ~5us of PE work
            # that runs while that transpose completes -- then close them.
            thunks = []
            for g in groups:
                thunks.extend(g[:3])
            for g in groups:
                thunks.append(g[3])
            return thunks

        # ---- global schedule ----
        # Two filler queues, consumed by fill() calls placed right after each
        # exp emission (the PE wait queue releases IN ORDER, so filler behind
        # a parked AV matmul cannot run early -- it must precede the AVs):
        #   gated:   qkv projection thunks + transposes; their execution also
        #            gates attention heads (per-pair markers).
        #   reserve: output-projection thunks, saved for the ACT-bound tail
        #            (the last quarter has the most exp work and the least
        #            attention-independent PE work).
        gated = []
        reserve = []
        gpos = [0]
        rpos = [0]

        def fill(k):
            take = min(k, len(gated) - gpos[0])
            for th in gated[gpos[0] : gpos[0] + take]:
                th()
            gpos[0] += take
            k -= take
            take = min(k, len(reserve) - rpos[0])
            for th in reserve[rpos[0] : rpos[0] + take]:
                th()
            rpos[0] += take

        def gate(idx):
            while gpos[0] < idx:
                gated[gpos[0]]()
                gpos[0] += 1

        pair_marker = {}

        def stage_qkv(tb, issue_dma=True):
            """Issue x prefetch + append quarter tb's projection thunks."""
            t0 = tb * TB
            if tb not in xh_tiles:
                nxh = xq_pool.tile([128, CC * TB], FP8, tag="xh", name=f"xh{tb}")
                nxl = xq_pool.tile([128, CC * TB], FP8, tag="xl", name=f"xl{tb}")
                xh_tiles[tb] = (nxh, nxl)
                nc.sync.dma_start(
                    nxh[:].rearrange("a (cc t) -> a cc t", cc=CC),
                    xTh.ap()[:, t0 : t0 + TB].rearrange("(cc a) t -> a cc t", a=128),
                )
                nc.sync.dma_start(
                    nxl[:].rearrange("a (cc t) -> a cc t", cc=CC),
                    xTl.ap()[:, t0 : t0 + TB].rearrange("(cc a) t -> a cc t", a=128),
                )
            qt_tiles[tb] = qt_pool.tile(
                [128, N_PAIRS * TB], BF16, tag="qT", name=f"qT{tb}"
            )
            base = len(gated)
            gated.extend(qkv_thunks(tb))
            n_v = (TB // 128) * 3 * NJ
            for p in range(N_PAIRS):
                # tb0 layout is [qk pairs..., v]; the v thunks there are not
                # score-gated (the AVs wait on v_buf semaphores and are
                # emitted after the whole v block)
                off = 0 if tb == 0 else n_v
                pair_marker[(tb, p)] = base + off + 2 * 3 * NJ * (p + 1)

        # The unit stream: per head, chunk-pair units; stage-1 (score+exp) of
        # unit i+1 is emitted BEFORE stage-2 (mask+AV) of unit i, so the exp
        # of the next pair runs on ScalarE while the PE processes the current
        # pair's AVs -- without this the in-order PE queue serializes
        # exp -> AV -> next scores -> next exp.
        #
        # Quarter 0 runs with an UNBOUNDED av-lag: all 16 tb0 units' scores +
        # exps are emitted before any mask/AV. The v-units (drawn as filler)
        # park on the startup DMA stream, and an emitted AV would park behind
        # them; deferring the AVs lets the exp train start as soon as the q/k
        # weights land (~10us) instead of waiting for the full v stream.
        stage_qkv(0, issue_dma=False)

        units = []
        for tbx in range(N_TB):
            for p in range(N_PAIRS):
                for h in range(2):
                    units.extend(head_units(tbx, p, h))

        started = set()
        pending_tp = []
        pending_av = []

        def pop_av():
            u = pending_av.pop(0)
            mask_av(u)
            if u["jj"] == u["n_chunk"] - 2:
                # head boundary: flush one pending transpose (its DVE
                # normalize dependency is a full head old by now)
                if pending_tp:
                    emit_transpose(*pending_tp.pop(0))
                if u["h"] == 1:
                    pending_tp.append((u["tb"], u["p"]))
                    if u["p"] == N_PAIRS - 1 and u["tb"] < N_TB - 1:
                        reserve.extend(proj_thunks(u["tb"]))

        for u in units:
            tbx, p, h = u["tb"], u["p"], u["h"]
            if u["jj"] == 0 and h == 0 and p == 0 and tbx not in started:
                started.add(tbx)
                if tbx + 1 < N_TB:
                    stage_qkv(tbx + 1)
                xh_tiles.pop(tbx, None)
                ynat_tiles[tbx] = ynat_pool.tile(
                    [128, N_TT * NCOL], BF16, tag="ynat", name=f"ynat{tbx}"
                )
                yt_tiles[tbx] = yt_pool.tile(
                    [128, N_PAIRS * TB], BF16, tag="yt", name=f"yt{tbx}"
                )
            if u["jj"] == 0:
                # pair p's q/k (and for tb>0, v) thunks must execute before
                # its scores
                gate(pair_marker[(tbx, p)])
            score_exp(u)
            fill(FILL_QUOTA[tbx][u["jj"] // 2])
            pending_av.append(u)
            lag = 99 if tbx == 0 else 1
            while len(pending_av) > lag:
                pop_av()
        while pending_av:
            pop_av()

        # drain remaining transposes and filler, then the final projection
        for tp_args in pending_tp:
            emit_transpose(*tp_args)
        gate(len(gated))
        fill(len(reserve) - rpos[0])
        for th in proj_thunks(N_TB - 1, borrow=True):
            th()

    nc.compile()
    return nc


_NC_CACHE = None


def kernel(x, Wq, bq, Wk, bk, Wv, bv, Wp, bp):
    global LAST_RESULTS, _NC_CACHE
    import ml_dtypes

    bf16 = ml_dtypes.bfloat16
    x = np.asarray(x, dtype=np.float32)
    Wq = np.asarray(Wq, dtype=np.float32)
    Wk = np.asarray(Wk, dtype=np.float32)
    Wv = np.asarray(Wv, dtype=np.float32)
    Wp = np.asarray(Wp, dtype=np.float32)
    bq = np.asarray(bq, dtype=np.float32)
    bk = np.asarray(bk, dtype=np.float32)
    bv = np.asarray(bv, dtype=np.float32)
    bp = np.asarray(bp, dtype=np.float32)

    if _NC_CACHE is None:
        _NC_CACHE = _build()
    nc = _NC_CACHE

    e4m3 = ml_dtypes.float8_e4m3

    def split8(a, s):
        """host split: a ~ hi/s + lo/s with hi = fp8(s*a), lo = fp8(s*a - hi);
        the kernel multiplies the raw fp8 values and descales the PSUM."""
        hi = (a * s).astype(e4m3)
        lo = ((a * s) - hi.astype(np.float32)).astype(e4m3)
        return np.ascontiguousarray(hi), np.ascontiguousarray(lo)

    scale = 1.0 / np.sqrt(D)
    # cores 2b and 2b+1 share x[b].T; cores with the same head-group share
    # the weight slices -- compute each unique tensor once
    xts = [split8(x[b].T, XS) for b in range(B)]
    wsets = []
    for hg in range(2):
        cols = slice(hg * NCOL, (hg + 1) * NCOL)
        qh, ql = split8(Wq[:, cols] * scale, QS)
        kh, kl = split8(Wk[:, cols], KS)
        vh, vl = split8(Wv[:, cols], VS)
        wsets.append(
            {
                "wqh": qh, "wql": ql,
                "wkh": kh, "wkl": kl,
                "wvh": vh, "wvl": vl,
                "wp": np.ascontiguousarray(Wp[cols, :]).astype(bf16),
                "bqk": np.ascontiguousarray(
                    np.stack([bq[cols] * scale, bk[cols]], axis=1)
                ).astype(np.float32),
                "bv": bv[cols].reshape(1, NCOL).astype(bf16),
            }
        )
    in_maps = [
        {"xTh": xts[core // 2][0], "xTl": xts[core // 2][1], **wsets[core % 2]}
        for core in range(8)
    ]

    res = run_bass_kernel_spmd(nc, in_maps, core_ids=list(range(8)), trace=TRACE)
    LAST_RESULTS = res

    result = np.empty((B, T, C), dtype=np.float32)
    for b in range(B):
        result[b] = (
            res.results[2 * b]["out"].astype(np.float32)
            + res.results[2 * b + 1]["out"].astype(np.float32)
            + bp
        )
    return result



# revision 55
# speedup vs baseline: 1.0017x; 1.0017x over previous
"""Causal self-attention (B=4, T=2048, C=1024, H=16, D=64) on 8 Trainium2 cores.

Sharding: core c = (b, hg) with b = c // 2 (batch), hg = c % 2 (head-group of
8 heads = 512 of 1024 qkv columns). Each core computes q/k/v projections for
its (b, hg), causal attention for its 8 heads, and a partial output
projection y_hg @ Wp[hg]. Host sums the two head-group partials per batch and
adds the projection bias.

Per-core kernel (scores/AV matmuls bf16, q/k/v projections in fp8e4m3
split-precision DoubleRow, PSUM accumulation + softmax in fp32):

  - q/k/v projections use the PE's fp8 DoubleRow mode (2 contraction k-tiles
    per instruction at 0.5 cycles/output-column = 4x the bf16 MAC rate).
    Accuracy is preserved with a 3-term split: host supplies xh = fp8(4*x),
    xl = fp8(4*x - xh) and Wh = fp8(S*W), Wl = fp8(S*W - Wh); the kernel
    accumulates xh@Wh + xh@Wl + xl@Wh in one PSUM group (the dropped xl@Wl
    term is ~0.05% of the result, below bf16 rounding). The descale constant
    1/(4*S) folds into the existing PSUM->SBUF bias-add (dual-scalar
    tensor_scalar / scalar_tensor_tensor), so DVE work is unchanged. This is
    MORE accurate than the bf16 baseline (measured) and 25% cheaper on PE.
  - qT/kT [col, t] via lhsT = weight chunk, rhs = xT chunk; v natural [t, col]
    via lhsT = xT chunk, rhs = Wv; v is stored in 65-wide groups per head with
    a ones-column.
  - scoresT chunks [s=128, t<=512] on PE, block-diagonal chunks packed
    compactly so one ScalarE exp instruction covers each chunk pair (no
    max-subtraction: logits are ~N(0,1); fp32 exp cannot overflow). Causal
    zeroing of diagonal chunks multiplies by a precomputed 0/1 mask (DVE).
  - AV runs TRANSPOSED for full PE-array utilization: lhsT = att chunk
    [s=128, t=128] (stationary), rhs = [v | ones] [s=128, 65] (moving),
    accumulating yz[t=128, 65] per (head, t-chunk) in PSUM; column 64 is the
    softmax denominator Z. One accumulation group per head (PSUM allows one
    open group per 2KB bank; first write per address overwrites). Rows are
    scaled by 1/Z into y natural [t, col] (bf16), then PE-transposed back to
    [col, t] for the output projection.

Scheduling: the attention stream is software-pipelined -- scores+exp of
chunk-pair i+1 are emitted BEFORE mask+AV of pair i, so ScalarE's exp always
overlaps PE work (the PE wait queue releases in order, so anything emitted
behind a parked AV cannot run early). Projection matmuls ride in two filler
queues consumed between pipeline stages: `gated` (q/k/v projections, whose
execution also gates attention heads via per-pair markers) and `reserve`
(output projections, saved for the exp-heaviest last quarter; Q1/Q2 quotas
deliberately under-draw so leftovers spill into Q3's exp shadow). Quarter 0
defers ALL its mask/AV work past the whole startup stream (q/k operands
first, v operands behind) so the exp train starts as soon as x+q/k weights
land (~12us); the deferred AVs flush under quarter 1's exp shadow. The final
projection opens all 8 PSUM output groups at once (two share each 2-bank
score buf -- accumulation groups are per-bank) so only the last pair's 8
closing matmuls wait on the final transpose, and the drain's psum->bf16
copies and output DMAs fan out over DVE/ACT and the SP/ACT/Pool DMA queues.
"""

import sys

if "/opt/trn_rl_repo" not in sys.path:
    sys.path.insert(0, "/opt/trn_rl_repo")

from contextlib import ExitStack

import numpy as np

import concourse.mybir as mybir
import concourse.tile as tile
from concourse import bacc, masks
from concourse.bass_utils import run_bass_kernel_spmd

F32 = mybir.dt.float32
BF16 = mybir.dt.bfloat16
FP8 = mybir.dt.float8e4
DR = mybir.MatmulPerfMode.DoubleRow
AF = mybir.ActivationFunctionType
MULT = mybir.AluOpType.mult
ADD = mybir.AluOpType.add

# split-precision scales: x staged as fp8(4x) + residual; weights staged as
# fp8(S*W) + residual with S chosen so values sit ~N(0,1) in e4m3 range.
# PSUM comes out 4*S too large; descale folds into the bias-add.
XS = 4.0
QS = 256.0   # wq carries the 1/sqrt(D) attention scale: sigma(256*Wq/8) = 1
KS = 32.0
VS = 32.0
QDESC = 1.0 / (XS * QS)
KDESC = 1.0 / (XS * KS)
VDESC = 1.0 / (XS * VS)

C = 1024      # embed dim
T = 2048      # sequence length
B = 4         # batch
NCOL = 512    # qkv columns per core (8 heads x 64)
TB = 512      # t-block / quarter size
SC = 128      # s-chunk size
D = 64        # head dim

N_WARM = 12  # PE clock-ramp dummy matmuls during startup DMA

LAST_RESULTS = None  # BassKernelResults of the most recent run (for test.py)
TRACE = False


def _build():
    N_PAIRS = NCOL // 128          # head-pairs per core (4)
    CC = C // 128                  # contraction chunks (8)
    N_TB = T // TB                 # t-blocks / quarters (4)
    SPB = TB // SC                 # s-chunks per t-block (4)
    N_TT = TB // SC                # t-subchunks per t-block (4)
    VGRP = 2 * N_PAIRS             # head groups in v_buf (8)
    VROW = VGRP * 65               # 520

    nc = bacc.Bacc("TRN2", target_bir_lowering=False, debug=False)

    xTh = nc.dram_tensor("xTh", (C, T), FP8, kind="ExternalInput")
    xTl = nc.dram_tensor("xTl", (C, T), FP8, kind="ExternalInput")
    wqh = nc.dram_tensor("wqh", (C, NCOL), FP8, kind="ExternalInput")
    wql = nc.dram_tensor("wql", (C, NCOL), FP8, kind="ExternalInput")
    wkh = nc.dram_tensor("wkh", (C, NCOL), FP8, kind="ExternalInput")
    wkl = nc.dram_tensor("wkl", (C, NCOL), FP8, kind="ExternalInput")
    wvh = nc.dram_tensor("wvh", (C, NCOL), FP8, kind="ExternalInput")
    wvl = nc.dram_tensor("wvl", (C, NCOL), FP8, kind="ExternalInput")
    wp = nc.dram_tensor("wp", (NCOL, C), BF16, kind="ExternalInput")
    bqk = nc.dram_tensor("bqk", (NCOL, 2), F32, kind="ExternalInput")
    bv = nc.dram_tensor("bv", (1, NCOL), BF16, kind="ExternalInput")
    out = nc.dram_tensor("out", (T, C), BF16, kind="ExternalOutput")

    with tile.TileContext(nc) as tc, ExitStack() as ctx:
        const = ctx.enter_context(tc.tile_pool(name="const", bufs=1))
        xq_pool = ctx.enter_context(tc.tile_pool(name="xq", bufs=4))
        w_pool = ctx.enter_context(tc.tile_pool(name="wqkv", bufs=1))
        qt_pool = ctx.enter_context(tc.tile_pool(name="qt", bufs=3))
        att_pool = ctx.enter_context(tc.tile_pool(name="att", bufs=20))
        ynat_pool = ctx.enter_context(tc.tile_pool(name="ynat", bufs=2))
        yt_pool = ctx.enter_context(tc.tile_pool(name="yt", bufs=4))
        small = ctx.enter_context(tc.tile_pool(name="small", bufs=2))
        ostage = ctx.enter_context(tc.tile_pool(name="ostage", bufs=8))
        ps_acc = ctx.enter_context(tc.tile_pool(name="ps_acc", bufs=2, space="PSUM"))
        ps1 = ps_acc
        ps_po = ps_acc
        ps_sc = ctx.enter_context(tc.tile_pool(name="ps_sc", bufs=2, space="PSUM"))
        ps_yz = ctx.enter_context(tc.tile_pool(name="ps_yz", bufs=2, space="PSUM"))

        kT = const.tile([128, N_PAIRS * T], BF16, tag="kT")   # [col_in_pair, p*T + s]
        v_buf = const.tile([128, (T // SC) * VROW], BF16, tag="vbuf")
        wp_sb = const.tile([128, N_PAIRS * C], BF16, tag="wp")
        bqk_sb = const.tile([128, 2 * N_PAIRS], F32, tag="bqk")
        bv_sb = const.tile([1, NCOL], BF16, tag="bv")
        wq_h = w_pool.tile([128, CC * NCOL], FP8, tag="wqh")
        wq_l = w_pool.tile([128, CC * NCOL], FP8, tag="wql")
        wk_h = w_pool.tile([128, CC * NCOL], FP8, tag="wkh")
        wk_l = w_pool.tile([128, CC * NCOL], FP8, tag="wkl")
        wv_h = w_pool.tile([128, CC * NCOL], FP8, tag="wvh")
        wv_l = w_pool.tile([128, CC * NCOL], FP8, tag="wvl")

        # Startup DMAs, all on ONE queue (the DMA device is serial and its
        # cross-queue arbitration is request-order FIFO, so a single queue is
        # the only way to get a deterministic priority order). The exp train
        # is the startup critical path, so q/k operands stream FIRST (x0h,
        # then hi-weight column halves for pairs 0/1, full lo weights), then
        # the v operands (consumed as deferred filler), then quarter 1's x
        # prefetch, then wp (first needed ~150us in).
        xh_tiles = {}
        x0h = xq_pool.tile([128, CC * TB], FP8, tag="xh", name="xh0")
        x0l = xq_pool.tile([128, CC * TB], FP8, tag="xl", name="xl0")
        xh_tiles[0] = (x0h, x0l)
        x1h = xq_pool.tile([128, CC * TB], FP8, tag="xh", name="xh1")
        x1l = xq_pool.tile([128, CC * TB], FP8, tag="xl", name="xl1")
        xh_tiles[1] = (x1h, x1l)
        xsrc_h = xTh.ap()[:, 0:TB].rearrange("(cc a) t -> a cc t", a=128)
        xsrc_l = xTl.ap()[:, 0:TB].rearrange("(cc a) t -> a cc t", a=128)
        wvh_src = wvh.ap().rearrange("(cc a) n -> a cc n", a=128)
        wvl_src = wvl.ap().rearrange("(cc a) n -> a cc n", a=128)
        nc.sync.dma_start(
            x0h[:].rearrange("a (cc t) -> a cc t", cc=CC), xsrc_h[:]
        )
        # hi q/k weights in column halves (fp8 256B rows pay the 2x DMA
        # latency penalty but pairs 0/1 start ~4us earlier); lo weights as
        # single full-width transfers (512B rows, full rate)
        HC = NCOL // 2
        def wdma(wsb, wsrc, half):
            nc.sync.dma_start(
                wsb[:].rearrange("a (cc n) -> a cc n", cc=CC)[
                    :, :, half * HC : (half + 1) * HC
                ],
                wsrc.ap().rearrange("(cc a) n -> a cc n", a=128)[
                    :, :, half * HC : (half + 1) * HC
                ],
            )
        wdma(wq_h, wqh, 0)
        nc.sync.dma_start(
            wq_l[:].rearrange("a (cc n) -> a cc n", cc=CC),
            wql.ap().rearrange("(cc a) n -> a cc n", a=128),
        )
        # x0l immediately after the q weights: the q/k units' third split
        # term (xl@Wh) gates the first scores
        nc.sync.dma_start(
            x0l[:].rearrange("a (cc t) -> a cc t", cc=CC), xsrc_l[:]
        )
        # merged q/k biases in ONE transfer (was two)
        nc.sync.dma_start(
            bqk_sb[:].rearrange("a (p o) -> a p o", o=2),
            bqk.ap().rearrange("(p a) o -> a p o", a=128),
        )
        wdma(wk_h, wkh, 0)
        nc.sync.dma_start(
            wk_l[:].rearrange("a (cc n) -> a cc n", cc=CC),
            wkl.ap().rearrange("(cc a) n -> a cc n", a=128),
        )
        wdma(wq_h, wqh, 1)
        wdma(wk_h, wkh, 1)
        nc.sync.dma_start(
            wv_h[:].rearrange("a (cc n) -> a cc n", cc=CC), wvh_src[:]
        )
        nc.sync.dma_start(bv_sb[:], bv.ap())
        nc.sync.dma_start(
            wv_l[:].rearrange("a (cc n) -> a cc n", cc=CC), wvl_src[:]
        )
        nc.sync.dma_start(
            x1h[:].rearrange("a (cc t) -> a cc t", cc=CC),
            xTh.ap()[:, TB : 2 * TB].rearrange("(cc a) t -> a cc t", a=128),
        )
        nc.sync.dma_start(
            x1l[:].rearrange("a (cc t) -> a cc t", cc=CC),
            xTl.ap()[:, TB : 2 * TB].rearrange("(cc a) t -> a cc t", a=128),
        )
        # wp is not needed until the first output projection (~60us in)
        nc.sync.dma_start(
            wp_sb[:].rearrange("a (p n) -> a p n", p=N_PAIRS),
            wp.ap().rearrange("(p a) n -> a p n", a=128),
        )
        # 0/1 causal triangle mask: msk[s, f] = (f >= s); block-diagonal
        # offset r uses the width-(TB - r*SC) prefix of the same tile.
        # memset to 1.0 first (cheap, DMA-independent) so the PE warm-up can
        # start immediately; the triangle select rewrites it in place after.
        msk = const.tile([128, TB], BF16, tag="msk")
        nc.vector.memset(msk[:, 0:256], 1.0)
        # PE warm-up: dummy matmuls on the DMA-independent mask tile keep the
        # PE clock ramped while the input DMAs stream; a guard read into an
        # unused cell keeps them alive through DCE
        warm_ps = ps_sc.tile([128, 2 * TB], F32, tag="st", name="warm_ps")
        for _ in range(N_WARM):
            nc.tensor.matmul(
                warm_ps[:, 0:256], msk[:, 0:128], msk[:, 0:256], start=True, stop=True
            )
        nc.vector.memset(msk[:, 256:TB], 1.0)
        nc.gpsimd.affine_select(
            out=msk[:],
            in_=msk[:],
            compare_op=mybir.AluOpType.is_ge,
            fill=0.0,
            base=0,
            channel_multiplier=-1,
            pattern=[[1, TB]],
        )
        guard = const.tile([1, 1], BF16, tag="guard")
        nc.vector.tensor_copy(guard[:], warm_ps[0:1, 0:1])
        nc.sync.dma_start(out.ap()[0:1, 0:1], guard[:])
        ones_f32 = const.tile([128, max(128, (T // SC) * VGRP)], F32, tag="ones_f32")
        nc.vector.memset(ones_f32[:], 1.0)
        # bv broadcast across partitions once: the v-units then fold the bias
        # into their psum->SBUF copy instead of spending a PE matmul each
        bv_bc = const.tile([128, NCOL], BF16, tag="bv_bc")
        nc.gpsimd.partition_broadcast(bv_bc[:], bv_sb[:])
        # ones columns of v_buf (col 64 of each 65-group)
        nc.vector.tensor_copy(
            v_buf[:].rearrange("a (t g o) -> a t g o", g=VGRP, o=65)[:, :, :, 64:65],
            ones_f32[:, : (T // SC) * VGRP].rearrange("a (t g) -> a t g", g=VGRP)[
                :, :, :, None
            ],
        )

        # per-pair filler draw: sized to the exp-vs-PE deficit of each quarter
        # so earlier quarters don't starve the ACT-bound last quarter
        # counts are in thunks; DR projection thunks are half the duration of
        # the baseline's bf16 thunks. Q0-Q2 pace the (cheap, 107ns) DR qkv
        # thunks; Q3's draws are deficit-sized for the (213ns) reserve proj
        # thunks so the reserve lasts through the exp-heavy last quarter.
        FILL_QUOTA = {
            0: [12, 12],
            1: [5, 5, 5, 5],
            2: [4, 4, 3, 3, 2, 2],
            3: [2, 0, 0, 2, 2, 2, 2, 2],
        }

        def head_units(tb, p, h):
            """Chunk-pair units of one attention head, for the global
            score/exp -> mask/AV software pipeline."""
            n_chunk = SPB * tb + SPB
            # diagonal chunks first: their exp->mask chain then overlaps with
            # the plain chunks' matmuls instead of stalling AV
            if tb > 0:
                j_order = (
                    [0, 1]
                    + list(range(SPB * tb, n_chunk))
                    + list(range(2, SPB * tb))
                )
            else:
                j_order = list(range(n_chunk))
            # per t-subchunk: positions in j_order of the first/last
            # contributing s-chunk (j contributes to tt iff j - SPB*tb <= tt)
            first_idx = {}
            last_idx = {}
            for idx, j in enumerate(j_order):
                r = j - SPB * tb
                for tt in range(max(r, 0), N_TT):
                    if tt not in first_idx:
                        first_idx[tt] = idx
                    last_idx[tt] = idx
            return [
                dict(
                    tb=tb, p=p, h=h, jj=jj, j_order=j_order, n_chunk=n_chunk,
                    first=first_idx, last=last_idx,
                )
                for jj in range(0, n_chunk, 2)
            ]

        def av_start_stop(u, idx, r, tt):
            """PSUM allows only ONE open accumulation group per 2KB bank
            (start marks the whole zero-region pending): the whole head's AV
            accumulation is a single group; the first write to each address
            overwrites, later writes accumulate."""
            r0 = u["j_order"][0] - SPB * u["tb"]
            start = idx == 0 and tt == max(r0, 0)
            stop = idx == u["n_chunk"] - 1 and tt == N_TT - 1
            return start, stop

        def score_exp(u):
            """Stage 1: score matmuls + exp for one chunk pair."""
            tb, p, h, jj = u["tb"], u["p"], u["h"], u["jj"]
            hrow = h * 64
            qT = qt_tiles[tb]
            st = ps_sc.tile([128, 2 * TB], F32, tag="st")
            at = att_pool.tile([128, 2 * TB], BF16, tag="at")
            # chunks are packed compactly (chunk k at offset o_k, width
            # TB - c0_k) so one exp instruction covers the whole pair
            cols = []
            o = 0
            for k in range(2):
                j = u["j_order"][jj + k]
                r = j - SPB * tb  # >=0 only for block-diag chunks
                c0 = max(0, r * SC)  # first valid t-col
                cols.append((jj + k, j, r, c0, o))
                nc.tensor.matmul(
                    st[:, o : o + TB - c0],
                    kT[hrow : hrow + 64, p * T + j * SC : p * T + j * SC + SC],
                    qT[hrow : hrow + 64, p * TB + c0 : (p + 1) * TB],
                    start=True,
                    stop=True,
                )
                o += TB - c0
            nc.scalar.activation(at[:, 0:o], st[:, 0:o], AF.Exp)
            u["at"] = at
            u["cols"] = cols

        head_yz = {}

        def mask_av(u):
            """Stage 2: causal mask + transposed-AV accumulation; emits the
            head's normalize after its last pair."""
            tb, p, h, jj = u["tb"], u["p"], u["h"], u["jj"]
            g = 2 * p + h
            at = u["at"]
            if jj == 0:
                head_yz[(tb, g)] = ps_yz.tile(
                    [128, N_TT * 65], F32, tag="yz", name=f"yz_{tb}_{g}"
                )
            yz = head_yz[(tb, g)]
            for idx, j, r, c0, o in u["cols"]:
                if r >= 0:
                    # zero att where t_loc < r*SC + s_loc
                    nc.vector.tensor_mul(
                        at[:, o : o + TB - c0],
                        at[:, o : o + TB - c0],
                        msk[:, 0 : TB - c0],
                    )
                vj = v_buf[:, j * VROW + g * 65 : j * VROW + g * 65 + 65]
                # transposed AV: att chunk stationary, [v | ones] moving;
                # accumulates yz[t, 0:64] = y and yz[t, 64] = Z
                for tt in range(max(r, 0), N_TT):
                    start, stop = av_start_stop(u, idx, r, tt)
                    nc.tensor.matmul(
                        yz[:, tt * 65 : tt * 65 + 65],
                        at[:, o + tt * SC - c0 : o + tt * SC - c0 + SC],
                        vj,
                        start=start,
                        stop=stop,
                    )
            if jj == u["n_chunk"] - 2:
                # normalize: y[t, d] / Z[t] for all 4 t-subchunks at once
                yzv = yz[:].rearrange("a (tt o) -> a tt o", o=65)
                rz = small.tile([128, N_TT], F32, tag="rz")
                nc.vector.reciprocal(rz[:][:, :, None], yzv[:, :, 64:65])
                ynat = ynat_tiles[tb]
                # pair-major ynat layout [t, (p, tt, 128)]: each pair's block
                # is contiguous, so ONE xbar DMA transposes it later
                nc.vector.tensor_mul(
                    ynat[:].rearrange(
                        "a (p tt c) -> a p tt c", p=N_PAIRS, tt=N_TT
                    )[:, p, :, h * 64 : h * 64 + 64],
                    yzv[:, :, 0:64],
                    rz[:][:, :, None].broadcast_to((128, N_TT, 64)),
                )
                head_yz.pop((tb, g))

        def emit_transpose(tb, p):
            """Transpose ynat[t, cols of pair p] -> yt[col, t] on the DMA
            xbar (out[c, tt, t] = in[t, tt*128+c]): one call per pair, 14ns
            per 16x128 tile on the ~17%-busy DMA device -- no PE matmuls, no
            DVE copies, no PSUM. Emitted a half-head after the normalize so
            the queue never parks on the input semaphore."""
            ynat = ynat_tiles[tb]
            yt = yt_tiles[tb]
            nc.sync.dma_start_transpose(
                yt[:, p * TB : (p + 1) * TB].rearrange(
                    "a (tt t) -> a tt t", tt=N_TT
                ),
                ynat[:, p * (N_TT * 128) : (p + 1) * (N_TT * 128)],
            )

        qt_tiles = {}
        ynat_tiles = {}
        yt_tiles = {}

        NJ = CC // 2  # DR k-tile pairs per contraction (4)

        def qkv_thunks(tb):
            """Per-matmul thunks for quarter tb's projections, to be spliced
            one-at-a-time into the attention stream of quarter tb-1.
            Ordered v-first, then (q, k) per pair, so att(tb) head pairs can
            start as soon as their own pair's projections are consumed.

            Each unit is 12 fp8 DoubleRow matmuls: 3 split terms (xh@Wh,
            xh@Wl, xl@Wh) x 4 k-tile pairs, one PSUM accumulation group."""
            thunks = []
            t0 = tb * TB
            xh8, xl8 = xh_tiles[tb]
            # (x tile, use-lo-weight): term order; start on the first, stop
            # on the last emitted matmul of the group
            TERMS = ((xh8, False), (xh8, True), (xl8, False))

            def wap(wsb, j, c0, c1):
                return wsb[:].rearrange("a (cc n) -> a cc n", cc=CC)[
                    :, 2 * j : 2 * j + 2, c0:c1
                ]

            def xap(xt, j, c0, c1):
                return xt[:].rearrange("a (cc t) -> a cc t", cc=CC)[
                    :, 2 * j : 2 * j + 2, c0:c1
                ]

            v_units = []
            for tth in range(TB // 128):
                tt = (t0 // 128) + tth
                pt_box = [None]
                # quarter 0 runs its v-units chunk-major behind the startup
                # DMA stream; units 2/3 borrow idle score-pool banks so all
                # four accumulation groups can be open at once
                vpool, vtag = (
                    (ps_yz, "yz") if tb == 0 and tth >= 2 else (ps1, "acc")
                )
                unit = []
                for ti, (xt, lo) in enumerate(TERMS):
                    wt = wv_l if lo else wv_h
                    def mkv(ti=ti, j=None, xt=xt, wt=wt, tth=tth, tt=tt,
                            pt_box=pt_box, vpool=vpool, vtag=vtag):
                        def go():
                            if ti == 0 and j == 0:
                                pt_box[0] = vpool.tile(
                                    [128, NCOL], F32, tag=vtag, name=f"psv_{tb}_{tth}"
                                )
                            pt = pt_box[0]
                            nc.tensor.matmul(
                                pt[:],
                                xap(xt, j, tth * 128, tth * 128 + 128),
                                wap(wt, j, 0, NCOL),
                                start=(ti == 0 and j == 0),
                                stop=(ti == 2 and j == NJ - 1),
                                perf_mode=DR,
                            )
                            if ti == 2 and j == NJ - 1:
                                nc.vector.scalar_tensor_tensor(
                                    v_buf[:, tt * VROW : (tt + 1) * VROW].rearrange(
                                        "a (g o) -> a g o", g=VGRP
                                    )[:, :, 0:64],
                                    pt[:].rearrange("a (g o) -> a g o", g=VGRP),
                                    VDESC,
                                    bv_bc[:].rearrange("a (g o) -> a g o", g=VGRP),
                                    MULT,
                                    ADD,
                                )
                        return go
                    unit.extend(mkv(j=j) for j in range(NJ))
                v_units.append(unit)
            v_thunks = []
            if tb == 0:
                # chunk-major: thunk i across units so each landed DMA chunk
                # group unblocks all four units' next matmul
                for i in range(3 * NJ):
                    for un in v_units:
                        v_thunks.append(un[i])
            else:
                for un in v_units:
                    v_thunks.extend(un)
            if tb > 0:
                # steady state: v first, then (q, k) per pair
                thunks.extend(v_thunks)
            for u in range(2 * N_PAIRS):
                p, which = u // 2, u % 2
                wh_sb, wl_sb, desc = (
                    (wq_h, wq_l, QDESC),
                    (wk_h, wk_l, KDESC),
                )[which]
                dst = (
                    qt_tiles[tb][:, p * TB : (p + 1) * TB]
                    if which == 0
                    else kT[:, p * T + t0 : p * T + t0 + TB]
                )
                pt_box = [None]
                for ti, (xt, lo) in enumerate(TERMS):
                    wt = wl_sb if lo else wh_sb
                    def mk(ti=ti, j=None, xt=xt, wt=wt, u=u, p=p, which=which,
                           desc=desc, dst=dst, pt_box=pt_box):
                        def go():
                            if ti == 0 and j == 0:
                                pt_box[0] = ps1.tile(
                                    [128, TB], F32, tag="acc", name=f"ps_{tb}_{u}"
                                )
                            pt = pt_box[0]
                            nc.tensor.matmul(
                                pt[:],
                                wap(wt, j, p * 128, p * 128 + 128),
                                xap(xt, j, 0, TB),
                                start=(ti == 0 and j == 0),
                                stop=(ti == 2 and j == NJ - 1),
                                perf_mode=DR,
                            )
                            if ti == 2 and j == NJ - 1:
                                nc.vector.tensor_scalar(
                                    dst, pt[:], desc,
                                    bqk_sb[:, 2 * p + which : 2 * p + which + 1],
                                    MULT, ADD,
                                )
                        return go
                    thunks.extend(mk(j=j) for j in range(NJ))
            if tb == 0:
                # quarter 0: q/k pairs FIRST so the exp train starts as soon
                # as the q/k weights land; the v block (startup-DMA paced)
                # rides behind as filler, with all tb0 AVs deferred past it
                thunks.extend(v_thunks)
            return thunks

        def proj_thunks(tb, borrow=False):
            """Per-matmul thunks for t-block tb's output projection. With
            borrow=True (the final projection, after all attention), half the
            accumulator groups use the idle score pool's slots so four
            groups pipeline instead of two."""
            t0 = tb * TB
            yt = yt_tiles[tb]
            groups = []
            sc_box = {}
            for tt in range(TB // 128):
                for nh in range(C // 512):
                    gi = tt * (C // 512) + nh
                    po_box = [None]
                    def mk(p, tt=tt, nh=nh, gi=gi, po_box=po_box):
                        def go():
                            if p == 0:
                                if borrow:
                                    # 8 concurrently-open groups: acc 2 +
                                    # yz 2 + two 2-bank sc bufs holding TWO
                                    # 512-col groups each (PSUM accumulation
                                    # groups are per-bank, so the two halves
                                    # are independent groups)
                                    kind = (0, 1, 1, 2, 0, 1, 1, 0)[gi]
                                    if kind == 1:
                                        if gi in (1, 5):
                                            sc_box[gi] = ps_sc.tile(
                                                [128, 1024], F32, tag="st",
                                                name=f"po_{tb}_{gi}",
                                            )
                                            po_box[0] = sc_box[gi][:, 0:512]
                                        else:
                                            po_box[0] = sc_box[gi - 1][:, 512:1024]
                                    elif kind == 2:
                                        po_box[0] = ps_yz.tile(
                                            [128, 512], F32, tag="yz",
                                            name=f"po_{tb}_{tt}_{nh}",
                                        )[:]
                                    else:
                                        po_box[0] = ps_po.tile(
                                            [128, 512], F32, tag="acc",
                                            name=f"po_{tb}_{tt}_{nh}",
                                        )[:]
                                else:
                                    po_box[0] = ps_po.tile(
                                        [128, 512], F32, tag="acc",
                                        name=f"po_{tb}_{tt}_{nh}",
                                    )[:]
                            po = po_box[0]
                            nc.tensor.matmul(
                                po,
                                yt[:, p * TB + tt * 128 : p * TB + tt * 128 + 128],
                                wp_sb[:, p * C + nh * 512 : p * C + nh * 512 + 512],
                                start=(p == 0),
                                stop=(p == N_PAIRS - 1),
                            )
                            if p == N_PAIRS - 1:
                                ob = ostage.tile([128, 512], BF16, tag="ob")
                                # psum->bf16 staging: copies split DVE/ACT
                                # (GPSIMD cannot read PSUM); each group's DMA
                                # issues from the queue OPPOSITE its copy
                                # engine so the four queues pipeline the tail
                                if borrow and gi % 2:
                                    nc.scalar.copy(ob[:], po)
                                else:
                                    nc.vector.tensor_copy(ob[:], po)
                                dq = (
                                    (nc.scalar, nc.sync, nc.gpsimd)[gi % 3]
                                    if borrow
                                    else nc.sync
                                )
                                dq.dma_start(
                                    out.ap()[
                                        t0 + tt * 128 : t0 + tt * 128 + 128,
                                        nh * 512 : (nh + 1) * 512,
                                    ],
                                    ob[:],
                                )
                        return go
                    groups.append([mk(p) for p in range(N_PAIRS)])
            if not borrow:
                return [th for g in groups for th in g]
            # Drain order: the last pair's yt block (p=3) depends on the
            # final transpose of the attention stream, so emit p0..p2 of all
            # eight concurrently-open PSUM groups first -- ~5us of PE work
            # that runs while that transpose completes -- then close them.
            thunks = []
            for g in groups:
                thunks.extend(g[:3])
            for g in groups:
                thunks.append(g[3])
            return thunks

        # ---- global schedule ----
        # Two filler queues, consumed by fill() calls placed right after each
        # exp emission (the PE wait queue releases IN ORDER, so filler behind
        # a parked AV matmul cannot run early -- it must precede the AVs):
        #   gated:   qkv projection thunks + transposes; their execution also
        #            gates attention heads (per-pair markers).
        #   reserve: output-projection thunks, saved for the ACT-bound tail
        #            (the last quarter has the most exp work and the least
        #            attention-independent PE work).
        gated = []
        reserve = []
        gpos = [0]
        rpos = [0]

        def fill(k):
            take = min(k, len(gated) - gpos[0])
            for th in gated[gpos[0] : gpos[0] + take]:
                th()
            gpos[0] += take
            k -= take
            take = min(k, len(reserve) - rpos[0])
            for th in reserve[rpos[0] : rpos[0] + take]:
                th()
            rpos[0] += take

        def gate(idx):
            while gpos[0] < idx:
                gated[gpos[0]]()
                gpos[0] += 1

        pair_marker = {}

        def stage_qkv(tb, issue_dma=True):
            """Issue x prefetch + append quarter tb's projection thunks."""
            t0 = tb * TB
            if tb not in xh_tiles:
                nxh = xq_pool.tile([128, CC * TB], FP8, tag="xh", name=f"xh{tb}")
                nxl = xq_pool.tile([128, CC * TB], FP8, tag="xl", name=f"xl{tb}")
                xh_tiles[tb] = (nxh, nxl)
                nc.sync.dma_start(
                    nxh[:].rearrange("a (cc t) -> a cc t", cc=CC),
                    xTh.ap()[:, t0 : t0 + TB].rearrange("(cc a) t -> a cc t", a=128),
                )
                nc.sync.dma_start(
                    nxl[:].rearrange("a (cc t) -> a cc t", cc=CC),
                    xTl.ap()[:, t0 : t0 + TB].rearrange("(cc a) t -> a cc t", a=128),
                )
            qt_tiles[tb] = qt_pool.tile(
                [128, N_PAIRS * TB], BF16, tag="qT", name=f"qT{tb}"
            )
            base = len(gated)
            gated.extend(qkv_thunks(tb))
            n_v = (TB // 128) * 3 * NJ
            for p in range(N_PAIRS):
                # tb0 layout is [qk pairs..., v]; the v thunks there are not
                # score-gated (the AVs wait on v_buf semaphores and are
                # emitted after the whole v block)
                off = 0 if tb == 0 else n_v
                pair_marker[(tb, p)] = base + off + 2 * 3 * NJ * (p + 1)

        # The unit stream: per head, chunk-pair units; stage-1 (score+exp) of
        # unit i+1 is emitted BEFORE stage-2 (mask+AV) of unit i, so the exp
        # of the next pair runs on ScalarE while the PE processes the current
        # pair's AVs -- without this the in-order PE queue serializes
        # exp -> AV -> next scores -> next exp.
        #
        # Quarter 0 runs with an UNBOUNDED av-lag: all 16 tb0 units' scores +
        # exps are emitted before any mask/AV. The v-units (drawn as filler)
        # park on the startup DMA stream, and an emitted AV would park behind
        # them; deferring the AVs lets the exp train start as soon as the q/k
        # weights land (~10us) instead of waiting for the full v stream.
        stage_qkv(0, issue_dma=False)

        units = []
        for tbx in range(N_TB):
            for p in range(N_PAIRS):
                for h in range(2):
                    units.extend(head_units(tbx, p, h))

        started = set()
        pending_tp = []
        pending_av = []

        def pop_av():
            u = pending_av.pop(0)
            mask_av(u)
            if u["jj"] == u["n_chunk"] - 2:
                # head boundary: flush one pending transpose (its DVE
                # normalize dependency is a full head old by now)
                if pending_tp:
                    emit_transpose(*pending_tp.pop(0))
                if u["h"] == 1:
                    pending_tp.append((u["tb"], u["p"]))
                    if u["p"] == N_PAIRS - 1 and u["tb"] < N_TB - 1:
                        reserve.extend(proj_thunks(u["tb"]))

        for u in units:
            tbx, p, h = u["tb"], u["p"], u["h"]
            if u["jj"] == 0 and h == 0 and p == 0 and tbx not in started:
                started.add(tbx)
                if tbx + 1 < N_TB:
                    stage_qkv(tbx + 1)
                xh_tiles.pop(tbx, None)
                ynat_tiles[tbx] = ynat_pool.tile(
                    [128, N_TT * NCOL], BF16, tag="ynat", name=f"ynat{tbx}"
                )
                yt_tiles[tbx] = yt_pool.tile(
                    [128, N_PAIRS * TB], BF16, tag="yt", name=f"yt{tbx}"
                )
            if u["jj"] == 0:
                # pair p's q/k (and for tb>0, v) thunks must execute before
                # its scores
                gate(pair_marker[(tbx, p)])
            score_exp(u)
            fill(FILL_QUOTA[tbx][u["jj"] // 2])
            pending_av.append(u)
            lag = 99 if tbx == 0 else 1
            while len(pending_av) > lag:
                pop_av()
        while pending_av:
            pop_av()

        # drain remaining transposes and filler, then the final projection
        for tp_args in pending_tp:
            emit_transpose(*tp_args)
        gate(len(gated))
        fill(len(reserve) - rpos[0])
        for th in proj_thunks(N_TB - 1, borrow=True):
            th()

    nc.compile()
    return nc


_NC_CACHE = None


def kernel(x, Wq, bq, Wk, bk, Wv, bv, Wp, bp):
    global LAST_RESULTS, _NC_CACHE
    import ml_dtypes

    bf16 = ml_dtypes.bfloat16
    x = np.asarray(x, dtype=np.float32)
    Wq = np.asarray(Wq, dtype=np.float32)
    Wk = np.asarray(Wk, dtype=np.float32)
    Wv = np.asarray(Wv, dtype=np.float32)
    Wp = np.asarray(Wp, dtype=np.float32)
    bq = np.asarray(bq, dtype=np.float32)
    bk = np.asarray(bk, dtype=np.float32)
    bv = np.asarray(bv, dtype=np.float32)
    bp = np.asarray(bp, dtype=np.float32)

    if _NC_CACHE is None:
        _NC_CACHE = _build()
    nc = _NC_CACHE

    e4m3 = ml_dtypes.float8_e4m3

    def split8(a, s):
        """host split: a ~ hi/s + lo/s with hi = fp8(s*a), lo = fp8(s*a - hi);
        the kernel multiplies the raw fp8 values and descales the PSUM."""
        hi = (a * s).astype(e4m3)
        lo = ((a * s) - hi.astype(np.float32)).astype(e4m3)
        return np.ascontiguousarray(hi), np.ascontiguousarray(lo)

    scale = 1.0 / np.sqrt(D)
    # cores 2b and 2b+1 share x[b].T; cores with the same head-group share
    # the weight slices -- compute each unique tensor once
    xts = [split8(x[b].T, XS) for b in range(B)]
    wsets = []
    for hg in range(2):
        cols = slice(hg * NCOL, (hg + 1) * NCOL)
        qh, ql = split8(Wq[:, cols] * scale, QS)
        kh, kl = split8(Wk[:, cols], KS)
        vh, vl = split8(Wv[:, cols], VS)
        wsets.append(
            {
                "wqh": qh, "wql": ql,
                "wkh": kh, "wkl": kl,
                "wvh": vh, "wvl": vl,
                "wp": np.ascontiguousarray(Wp[cols, :]).astype(bf16),
                "bqk": np.ascontiguousarray(
                    np.stack([bq[cols] * scale, bk[cols]], axis=1)
                ).astype(np.float32),
                "bv": bv[cols].reshape(1, NCOL).astype(bf16),
            }
        )
    in_maps = [
        {"xTh": xts[core // 2][0], "xTl": xts[core // 2][1], **wsets[core % 2]}
        for core in range(8)
    ]

    res = run_bass_kernel_spmd(nc, in_maps, core_ids=list(range(8)), trace=TRACE)
    LAST_RESULTS = res

    result = np.empty((B, T, C), dtype=np.float32)
    for b in range(B):
        result[b] = (
            res.results[2 * b]["out"].astype(np.float32)
            + res.results[2 * b + 1]["out"].astype(np.float32)
            + bp
        )
    return result

